# revision 6
# baseline (speedup 1.0000x reference)
"""HartleyCosineConv2d — per-channel symmetric 7-tap conv along H with reflect
padding, as per-channel banded-matrix matmuls on the TensorEngine.

y[n,c,i,w] = sum_s w[c,s] * x[n,c,reflect(i-s),w],  s in {0,±1,±2,±3},
tied-symmetric coefficients alpha[c, 0..3].

Formulation: y[n,c] = A_c @ x[n,c] where A_c (H x H) is a 7-diagonal banded
matrix (+ reflect folds in the first/last 3 rows), built on the host from
alpha. H=256 is blocked 2x128: per channel, two diagonal 128x128 blocks plus
two 3-row corner-coupling blocks, all executed as PE matmuls (float32r,
1 cyc/row) accumulating in PSUM.

Sharding: channel-parallel — core k handles channels [32k, 32k+32) for all 8
batches, so per-core weight traffic is only ~4 MB vs 64 MB of x.
"""

import numpy as np

import concourse.mybir as mybir
from concourse import bacc, bass_utils
from concourse.tile import TileContext

# Problem shapes (hardcoded per contract).
N, C, H, W = 8, 256, 256, 256
M = 3  # max shift
NCORES = 8
CS = C // NCORES  # channels per core
HB = H // 2  # H block = 128 partitions
CHUNK = 8  # channels processed per (batch, chunk) iteration
NCHUNK = CS // CHUNK

DT_MM = mybir.dt.float32r  # matmul operand dtype (f32 layout, 1 cyc/row)
DT_F32 = mybir.dt.float32


def _reflect(h, s):
    i = np.arange(h)
    idx = i - s
    idx = np.where(idx < 0, -idx, idx)
    idx = np.where(idx >= h, 2 * h - 2 - idx, idx)
    return idx


def _build_weights(alpha):
    """alpha (C, 1+M) -> wm (HB, C, 2, HB), wc (M, C, 2, HB), layout [j, c, b, i]:
    wm[j,c,b,i] = A_c[b*HB+i, b*HB+j]   (diagonal blocks, transposed for lhsT)
    wc[j,c,0,i] = A_c[i, HB+j]          (rows 125..127 <- inputs 128..130)
    wc[j,c,1,i] = A_c[HB+i, HB-M+j]     (rows 128..130 <- inputs 125..127)
    """
    c_, km = alpha.shape
    assert (c_, km) == (C, 1 + M)
    shifts = [0]
    for m in range(1, M + 1):
        shifts.extend([m, -m])
    A = np.zeros((C, H, H), np.float32)
    i = np.arange(H)
    for s in shifts:
        coef = alpha[:, abs(s)]
        idx = _reflect(H, s)
        A[:, i, idx] += coef[:, None]
    wm = np.empty((HB, C, 2, HB), np.float32)
    wm[:, :, 0, :] = A[:, 0:HB, 0:HB].transpose(2, 0, 1)
    wm[:, :, 1, :] = A[:, HB:H, HB:H].transpose(2, 0, 1)
    wc = np.empty((M, C, 2, HB), np.float32)
    wc[:, :, 0, :] = A[:, 0:HB, HB : HB + M].transpose(2, 0, 1)
    wc[:, :, 1, :] = A[:, HB:H, HB - M : HB].transpose(2, 0, 1)
    return wm, wc


def _build_nc():
    nc = bacc.Bacc("TRN2", target_bir_lowering=False)
    xs = nc.dram_tensor("xs", [N, CS, H, W], DT_MM, kind="ExternalInput")
    wm = nc.dram_tensor("wm", [HB, CS, 2, HB], DT_MM, kind="ExternalInput")
    wc = nc.dram_tensor("wc", [M, CS, 2, HB], DT_MM, kind="ExternalInput")
    ys = nc.dram_tensor("ys", [N, CS, H, W], DT_F32, kind="ExternalOutput")

    with TileContext(nc) as tc:
        with (
            tc.tile_pool(name="wpool", bufs=1) as wpool,
            tc.tile_pool(name="xin", bufs=3) as xpool,
            tc.tile_pool(name="halo", bufs=3) as hpool,
            tc.tile_pool(name="yout", bufs=3) as ypool,
            tc.tile_pool(name="ps", bufs=8, space="PSUM") as pp,
        ):
            wm_sb = wpool.tile([HB, CS * 2 * HB], DT_MM)
            nc.sync.dma_start(out=wm_sb, in_=wm.rearrange("j c b i -> j (c b i)"))
            wc_sb = wpool.tile([M, CS * 2 * HB], DT_MM)
            nc.sync.dma_start(out=wc_sb, in_=wc.rearrange("j c b i -> j (c b i)"))

            for n in range(N):
                for ch in range(NCHUNK):
                    c0 = ch * CHUNK
                    # x[n, c0:c0+CHUNK] as [h-in-block partitions, (c, block, w)]
                    xin = xpool.tile([HB, CHUNK * 2 * W], DT_MM, tag="xin")
                    nc.sync.dma_start(
                        out=xin.rearrange("p (c b w) -> p c b w", c=CHUNK, b=2),
                        in_=xs[n, c0 : c0 + CHUNK].rearrange(
                            "c (b p) w -> p c b w", p=HB
                        ),
                    )
                    # bottom halo: x rows 125..127 per channel, at partitions 0..2
                    hbot = hpool.tile([M, CHUNK * W], DT_MM, tag="hbot")
                    nc.sync.dma_start(
                        out=hbot.rearrange("p (c w) -> p c w", c=CHUNK),
                        in_=xs[n, c0 : c0 + CHUNK, HB - M : HB, :].rearrange(
                            "c p w -> p c w"
                        ),
                    )
                    yout = ypool.tile([HB, CHUNK * 2 * W], DT_F32, tag="yout")
                    for cc in range(CHUNK):
                        c = c0 + cc
                        f0 = cc * 2 * W  # free offset of this channel's block 0
                        ps0 = pp.tile([HB, W], DT_F32, tag="ps")
                        ps1 = pp.tile([HB, W], DT_F32, tag="ps")
                        lhs0 = wm_sb[:, (c * 2 + 0) * HB : (c * 2 + 1) * HB]
                        lhs1 = wm_sb[:, (c * 2 + 1) * HB : (c * 2 + 2) * HB]
                        lc0 = wc_sb[:, (c * 2 + 0) * HB : (c * 2 + 1) * HB]
                        lc1 = wc_sb[:, (c * 2 + 1) * HB : (c * 2 + 2) * HB]
                        # block 0: diag + coupling from x rows 128..130
                        nc.tensor.matmul(
                            ps0, lhs0, xin[:, f0 : f0 + W], start=True, stop=False
                        )
                        nc.tensor.matmul(
                            ps0,
                            lc0,
                            xin[0:M, f0 + W : f0 + 2 * W],
                            start=False,
                            stop=True,
                        )
                        # block 1: diag + coupling from x rows 125..127
                        nc.tensor.matmul(
                            ps1, lhs1, xin[:, f0 + W : f0 + 2 * W], start=True, stop=False
                        )
                        nc.tensor.matmul(
                            ps1,
                            lc1,
                            hbot[:, cc * W : (cc + 1) * W],
                            start=False,
                            stop=True,
                        )
                        # evacuate PSUM -> SBUF, split across ACT and DVE
                        nc.scalar.copy(out=yout[:, f0 : f0 + W], in_=ps0)
                        nc.vector.tensor_copy(
                            out=yout[:, f0 + W : f0 + 2 * W], in_=ps1
                        )
                    nc.sync.dma_start(
                        out=ys[n, c0 : c0 + CHUNK].rearrange(
                            "c (b p) w -> p c b w", p=HB
                        ),
                        in_=yout.rearrange("p (c b w) -> p c b w", c=CHUNK, b=2),
                    )
    nc.compile()
    return nc


_NC_CACHE = None


def _get_nc():
    global _NC_CACHE
    if _NC_CACHE is None:
        _NC_CACHE = _build_nc()
    return _NC_CACHE


def _make_in_maps(x, alpha):
    x = np.ascontiguousarray(np.asarray(x), dtype=np.float32)
    alpha = np.ascontiguousarray(np.asarray(alpha), dtype=np.float32)
    wm, wc = _build_weights(alpha)
    in_maps = []
    for k in range(NCORES):
        c0 = k * CS
        in_maps.append(
            {
                "xs": np.ascontiguousarray(x[:, c0 : c0 + CS]),
                "wm": np.ascontiguousarray(wm[:, c0 : c0 + CS]),
                "wc": np.ascontiguousarray(wc[:, c0 : c0 + CS]),
            }
        )
    return in_maps


def run_spmd(x, alpha, **kwargs):
    """Run the bass kernel on 8 cores; returns (y, BassKernelResults)."""
    nc = _get_nc()
    in_maps = _make_in_maps(x, alpha)
    res = bass_utils.run_bass_kernel_spmd(
        nc, in_maps, core_ids=list(range(NCORES)), **kwargs
    )
    y = np.concatenate([r["ys"] for r in res.results], axis=1)
    return y, res


def kernel(x, alpha):
    y, _ = run_spmd(x, alpha)
    return y


# revision 9
# speedup vs baseline: 213.2849x; 213.2849x over previous
"""HartleyCosineConv2d — per-channel symmetric 7-tap conv along H with reflect
padding, as per-channel banded-matrix matmuls on the TensorEngine.

y[n,c,i,w] = sum_s w[c,s] * x[n,c,reflect(i-s),w],  s in {0,±1,±2,±3},
tied-symmetric coefficients alpha[c, 0..3].

Formulation: y[n,c] = A_c @ x[n,c] where A_c (H x H) is a 7-diagonal banded
matrix (+ reflect folds in the first/last 3 rows), built on the host from
alpha. H=256 is blocked 2x128: per channel, two diagonal 128x128 blocks plus
two 3-row corner-coupling blocks, all executed as PE matmuls (float32r,
1 cyc/row) accumulating in PSUM.

Sharding: channel-parallel — core k handles channels [32k, 32k+32) for all 8
batches, so per-core weight traffic is only ~4 MB vs 64 MB of x.
"""

import numpy as np

import concourse.mybir as mybir
from concourse import bacc, bass_utils
from concourse.tile import TileContext

# Problem shapes (hardcoded per contract).
N, C, H, W = 8, 256, 256, 256
M = 3  # max shift
NCORES = 8
CS = C // NCORES  # channels per core
HB = H // 2  # H block = 128 partitions
CHUNK = 8  # channels processed per (batch, chunk) iteration
NCHUNK = CS // CHUNK

DT_MM = mybir.dt.float32r  # matmul operand dtype (f32 layout, 1 cyc/row)
DT_F32 = mybir.dt.float32


def _reflect(h, s):
    i = np.arange(h)
    idx = i - s
    idx = np.where(idx < 0, -idx, idx)
    idx = np.where(idx >= h, 2 * h - 2 - idx, idx)
    return idx


def _build_weights(alpha):
    """alpha (C, 1+M) -> wm (HB, C, 2, HB), wc (M, C, 2, HB), layout [j, c, b, i]:
    wm[j,c,b,i] = A_c[b*HB+i, b*HB+j]   (diagonal blocks, transposed for lhsT)
    wc[j,c,0,i] = A_c[i, HB+j]          (rows 125..127 <- inputs 128..130)
    wc[j,c,1,i] = A_c[HB+i, HB-M+j]     (rows 128..130 <- inputs 125..127)
    """
    c_, km = alpha.shape
    assert (c_, km) == (C, 1 + M)
    shifts = [0]
    for m in range(1, M + 1):
        shifts.extend([m, -m])
    A = np.zeros((C, H, H), np.float32)
    i = np.arange(H)
    for s in shifts:
        coef = alpha[:, abs(s)]
        idx = _reflect(H, s)
        A[:, i, idx] += coef[:, None]
    wm = np.empty((HB, C, 2, HB), np.float32)
    wm[:, :, 0, :] = A[:, 0:HB, 0:HB].transpose(2, 0, 1)
    wm[:, :, 1, :] = A[:, HB:H, HB:H].transpose(2, 0, 1)
    wc = np.empty((M, C, 2, HB), np.float32)
    wc[:, :, 0, :] = A[:, 0:HB, HB : HB + M].transpose(2, 0, 1)
    wc[:, :, 1, :] = A[:, HB:H, HB - M : HB].transpose(2, 0, 1)
    return wm, wc


def _build_nc(repeat=1):
    nc = bacc.Bacc("TRN2", target_bir_lowering=False)
    xs = nc.dram_tensor("xs", [N, CS, H, W], DT_MM, kind="ExternalInput")
    wm = nc.dram_tensor("wm", [HB, CS, 2, HB], DT_MM, kind="ExternalInput")
    wc = nc.dram_tensor("wc", [M, CS, 2, HB], DT_MM, kind="ExternalInput")
    ys = nc.dram_tensor("ys", [N, CS, H, W], DT_F32, kind="ExternalOutput")

    with TileContext(nc) as tc:
        with (
            tc.tile_pool(name="wpool", bufs=1) as wpool,
            tc.tile_pool(name="xin", bufs=3) as xpool,
            tc.tile_pool(name="halo", bufs=3) as hpool,
            tc.tile_pool(name="yout", bufs=3) as ypool,
            tc.tile_pool(name="ps", bufs=8, space="PSUM") as pp,
        ):
            wm_sb = wpool.tile([HB, CS * 2 * HB], DT_MM)
            nc.sync.dma_start(out=wm_sb, in_=wm.rearrange("j c b i -> j (c b i)"))
            wc_sb = wpool.tile([M, CS * 2 * HB], DT_MM)
            nc.sync.dma_start(out=wc_sb, in_=wc.rearrange("j c b i -> j (c b i)"))

            import contextlib

            rep_ctx = (
                tc.For_i(0, repeat, 1) if repeat > 1 else contextlib.nullcontext()
            )
            with rep_ctx:
                _emit_body(nc, tc, xpool, hpool, ypool, pp, xs, ys, wm_sb, wc_sb)
    nc.compile()
    return nc


def _emit_body(nc, tc, xpool, hpool, ypool, pp, xs, ys, wm_sb, wc_sb):
    if True:
        if True:
            for n in range(N):
                for ch in range(NCHUNK):
                    c0 = ch * CHUNK
                    # x[n, c0:c0+CHUNK] as [h-in-block partitions, (c, block, w)]
                    xin = xpool.tile([HB, CHUNK * 2 * W], DT_MM, tag="xin")
                    nc.sync.dma_start(
                        out=xin.rearrange("p (c b w) -> p c b w", c=CHUNK, b=2),
                        in_=xs[n, c0 : c0 + CHUNK].rearrange(
                            "c (b p) w -> p c b w", p=HB
                        ),
                    )
                    # bottom halo: x rows 125..127 per channel, at partitions 0..2
                    hbot = hpool.tile([M, CHUNK * W], DT_MM, tag="hbot")
                    nc.sync.dma_start(
                        out=hbot.rearrange("p (c w) -> p c w", c=CHUNK),
                        in_=xs[n, c0 : c0 + CHUNK, HB - M : HB, :].rearrange(
                            "c p w -> p c w"
                        ),
                    )
                    yout = ypool.tile([HB, CHUNK * 2 * W], DT_F32, tag="yout")
                    for cc in range(CHUNK):
                        c = c0 + cc
                        f0 = cc * 2 * W  # free offset of this channel's block 0
                        ps0 = pp.tile([HB, W], DT_F32, tag="ps")
                        ps1 = pp.tile([HB, W], DT_F32, tag="ps")
                        lhs0 = wm_sb[:, (c * 2 + 0) * HB : (c * 2 + 1) * HB]
                        lhs1 = wm_sb[:, (c * 2 + 1) * HB : (c * 2 + 2) * HB]
                        lc0 = wc_sb[:, (c * 2 + 0) * HB : (c * 2 + 1) * HB]
                        lc1 = wc_sb[:, (c * 2 + 1) * HB : (c * 2 + 2) * HB]
                        # block 0: diag + coupling from x rows 128..130
                        nc.tensor.matmul(
                            ps0, lhs0, xin[:, f0 : f0 + W], start=True, stop=False
                        )
                        nc.tensor.matmul(
                            ps0,
                            lc0,
                            xin[0:M, f0 + W : f0 + 2 * W],
                            start=False,
                            stop=True,
                        )
                        # block 1: diag + coupling from x rows 125..127
                        nc.tensor.matmul(
                            ps1, lhs1, xin[:, f0 + W : f0 + 2 * W], start=True, stop=False
                        )
                        nc.tensor.matmul(
                            ps1,
                            lc1,
                            hbot[:, cc * W : (cc + 1) * W],
                            start=False,
                            stop=True,
                        )
                        # evacuate PSUM -> SBUF, split across ACT and DVE
                        nc.scalar.copy(out=yout[:, f0 : f0 + W], in_=ps0)
                        nc.vector.tensor_copy(
                            out=yout[:, f0 + W : f0 + 2 * W], in_=ps1
                        )
                    nc.sync.dma_start(
                        out=ys[n, c0 : c0 + CHUNK].rearrange(
                            "c (b p) w -> p c b w", p=HB
                        ),
                        in_=yout.rearrange("p (c b w) -> p c b w", c=CHUNK, b=2),
                    )


_NC_CACHE = {}


def _get_nc(repeat=1):
    if repeat not in _NC_CACHE:
        _NC_CACHE[repeat] = _build_nc(repeat)
    return _NC_CACHE[repeat]


def _make_in_maps(x, alpha):
    x = np.ascontiguousarray(np.asarray(x), dtype=np.float32)
    alpha = np.ascontiguousarray(np.asarray(alpha), dtype=np.float32)
    wm, wc = _build_weights(alpha)
    in_maps = []
    for k in range(NCORES):
        c0 = k * CS
        in_maps.append(
            {
                "xs": np.ascontiguousarray(x[:, c0 : c0 + CS]),
                "wm": np.ascontiguousarray(wm[:, c0 : c0 + CS]),
                "wc": np.ascontiguousarray(wc[:, c0 : c0 + CS]),
            }
        )
    return in_maps


def run_spmd(x, alpha, **kwargs):
    """Run the bass kernel on 8 cores; returns (y, BassKernelResults)."""
    nc = _get_nc()
    in_maps = _make_in_maps(x, alpha)
    res = bass_utils.run_bass_kernel_spmd(
        nc, in_maps, core_ids=list(range(NCORES)), **kwargs
    )
    y = np.concatenate([r["ys"] for r in res.results], axis=1)
    return y, res


def kernel(x, alpha):
    y, _ = run_spmd(x, alpha)
    return y
